# revision 12
# baseline (speedup 1.0000x reference)
"""DeepseekV2 MoE layer on 8 Trainium2 NeuronCores (Bass/Tile).

Strategy (expert-parallel, per sharding hint):
  - Router (softmax + top-6 + renormalize) computed host-side in fp64;
    it is 67 MFLOP of a 106 GFLOP layer but serializes the device
    pipeline, so the host precomputes the dispatch instead:
      * experts greedy-paired 2-per-core to balance token counts,
      * per-core gathered activations x^T[:, selected tokens] are
        pre-tiled host-side (slot-major, expert blocks A|B),
      * combine weights are folded into a per-core scatter one-hot
        P_w[slot, token] (bf16) built host-side.
  - Device per core: SwiGLU over its ~800 slots (bf16, f32 psum),
    shared-expert SwiGLU sharded 8-way over the intermediate dim,
    down-projections + scatter-combine accumulated in one PSUM group
    per token tile, ReduceScatter (bf16) per 512-column block.
  - Scatter matmuls are emitted only for (slot-tile, token-tile) pairs
    that are nonzero on at least one core (slots are token-sorted
    within an expert, so each slot tile touches ~3-4 token tiles).

All weights are pre-tiled host-side into the exact SBUF layouts so
every device DMA is a few large contiguous descriptors.
"""

import numpy as np
import ml_dtypes

import concourse.bass as bass  # noqa: F401  (AP types)
import concourse.mybir as mybir
import concourse.tile as tile
from concourse import bacc
from concourse import bass_utils
from concourse.bass_interp import get_hw_module

F32 = mybir.dt.float32
BF16 = mybir.dt.bfloat16
ALU = mybir.AluOpType
ACTF = mybir.ActivationFunctionType

T = 1024      # tokens
H = 2048      # hidden
I = 1408      # moe intermediate
E = 16        # routed experts
K = 6         # experts per token
SI = 2816     # shared intermediate
NC = 8        # cores
NHC = H // 128           # 16 h chunks
NIT = I // 128           # 11 routed i tiles
SIL = SI // NC           # shared intermediate per core (352)
SH_M = [128, 128, 96]    # shared i tile heights
HB = 512                 # stage-B h block == ReduceScatter chunk
NHB = H // HB            # 4
NTT = T // 128           # 8 token tiles
TB = 512                 # stage-A token block for the shared expert
NTB = T // TB            # 2
BF = ml_dtypes.bfloat16


def _route(x, gate_w):
    """Exact router in fp64: comb[t, e] = renormalized top-6 softmax weight."""
    xl = np.asarray(x, np.float64)
    logits = xl @ np.asarray(gate_w, np.float64).T
    logits -= logits.max(-1, keepdims=True)
    ex = np.exp(logits)
    probs = ex / ex.sum(-1, keepdims=True)
    idx = np.argsort(-probs, axis=-1, kind="stable")[:, :K]
    topw = np.take_along_axis(probs, idx, axis=-1)
    topw = topw / topw.sum(-1, keepdims=True)
    comb = np.zeros((T, E))
    np.put_along_axis(comb, idx, topw, axis=-1)
    return comb


def _plan(comb):
    """Pair experts 2-per-core (big with small) and fix slot capacities."""
    counts = (comb > 0).sum(0)
    order = np.argsort(-counts, kind="stable")
    pairs = [(int(order[r]), int(order[2 * NC - 1 - r])) for r in range(NC)]
    cap_a = int(max(counts[a] for a, _ in pairs))
    cap_b = int(max(counts[b] for _, b in pairs))
    # slot tiles: expert block A at cols [0, cap_a), block B at [cap_a, ns)
    tiles = []  # (j, col_offset, width)
    for j, cap, off in ((0, cap_a, 0), (1, cap_b, cap_a)):
        for t0 in range(0, cap, 128):
            tiles.append((j, off + t0, min(128, cap - t0)))
    return pairs, cap_a, cap_b, tiles


def _build_program(cap_a, cap_b, tiles, adj):
    """adj[ti] = sorted list of token-tile indices with any nonzero P block."""
    ns = cap_a + cap_b
    nt = len(tiles)
    nc = bacc.Bacc("TRN2", target_bir_lowering=False, debug=False,
                   enable_asserts=False, num_devices=NC)

    xTr_d = nc.dram_tensor("xTr", [NTB, 128, NHC, TB], BF16, kind="ExternalInput")
    xgT_d = nc.dram_tensor("xgT", [128, NHC, ns], BF16, kind="ExternalInput")
    pw_d = nc.dram_tensor("pw", [128, nt, T], BF16, kind="ExternalInput")
    wg_d = nc.dram_tensor("wg", [2, NIT, 128, NHC, 128], BF16, kind="ExternalInput")
    wu_d = nc.dram_tensor("wu", [2, NIT, 128, NHC, 128], BF16, kind="ExternalInput")
    wd_d = nc.dram_tensor("wd", [2, NHB, 128, NIT, HB], BF16, kind="ExternalInput")
    swg_d = nc.dram_tensor("swg", [len(SH_M), 128, NHC, 128], BF16,
                           kind="ExternalInput")
    swu_d = nc.dram_tensor("swu", [len(SH_M), 128, NHC, 128], BF16,
                           kind="ExternalInput")
    swd_d = nc.dram_tensor("swd", [NHB, 128, len(SH_M), HB], BF16,
                           kind="ExternalInput")
    out_d = nc.dram_tensor("out", [NHB, T // NC, HB], BF16, kind="ExternalOutput")

    import contextlib
    with tile.TileContext(nc) as tc, contextlib.ExitStack() as st:
        xin_pool = st.enter_context(tc.tile_pool(name="xin", bufs=1))
        ch_pool = st.enter_context(tc.tile_pool(name="ch", bufs=1))
        wgu_pool = st.enter_context(tc.tile_pool(name="wgu", bufs=2))
        wd_pool = st.enter_context(tc.tile_pool(name="wd", bufs=2))
        y_pool = st.enter_context(tc.tile_pool(name="yb", bufs=1))
        act_pool = st.enter_context(tc.tile_pool(name="act", bufs=2))
        ob_pool = st.enter_context(tc.tile_pool(name="ob", bufs=3))
        psa_pool = st.enter_context(tc.tile_pool(name="psa", bufs=2, space="PSUM"))
        psb_pool = st.enter_context(tc.tile_pool(name="psb", bufs=2, space="PSUM"))
        dram_pool = st.enter_context(tc.tile_pool(name="dram", bufs=1, space="DRAM"))

        # ---- resident inputs, ordered so first-needed data lands first ----
        xTr = xin_pool.tile([128, NTB, NHC, TB], BF16, tag="xTr")
        nc.sync.dma_start(xTr[:, 0], xTr_d[0])
        swg = [xin_pool.tile([128, NHC, 128], BF16, tag=f"swg{s}",
                             name=f"swg{s}") for s in range(len(SH_M))]
        swu = [xin_pool.tile([128, NHC, 128], BF16, tag=f"swu{s}",
                             name=f"swu{s}") for s in range(len(SH_M))]
        for s in range(len(SH_M)):
            nc.sync.dma_start(swg[s][:], swg_d[s])
            nc.sync.dma_start(swu[s][:], swu_d[s])
        nc.sync.dma_start(xTr[:, 1], xTr_d[1])
        xgT = xin_pool.tile([128, NHC, ns], BF16, tag="xgT")
        nc.sync.dma_start(xgT[:], xgT_d[:])

        ch_sh = [ch_pool.tile([128, T], BF16, tag=f"chs{s}", name=f"chs{s}")
                 for s in range(len(SH_M))]
        ch_rt = [ch_pool.tile([128, ns], BF16, tag=f"chr{it}", name=f"chr{it}")
                 for it in range(NIT)]

        # ---- stage A: shared expert SwiGLU (all tokens, SIL slice) ----
        for tb in range(NTB):
            for sit, m in enumerate(SH_M):
                psg = psa_pool.tile([128, 512], F32, tag="psg")
                psu = psa_pool.tile([128, 512], F32, tag="psu")
                for hc in range(NHC):
                    nc.tensor.matmul(psg[:m], swg[sit][:, hc, :m],
                                     xTr[:, tb, hc, :],
                                     start=(hc == 0), stop=(hc == NHC - 1))
                for hc in range(NHC):
                    nc.tensor.matmul(psu[:m], swu[sit][:, hc, :m],
                                     xTr[:, tb, hc, :],
                                     start=(hc == 0), stop=(hc == NHC - 1))
                sg = act_pool.tile([128, 512], F32, tag="sg")
                nc.scalar.activation(sg[:m], psg[:m], ACTF.Silu)
                nc.vector.tensor_mul(ch_sh[sit][:m, tb * TB:(tb + 1) * TB],
                                     sg[:m], psu[:m])

        # ---- stage B weight prefetch plumbing ----
        # stage-B weights go through the Activation engine's HWDGE queues so
        # the paced gate/up stream on the SP queues can't head-of-line block
        # them (SP queue entries stall at the head waiting for pool slots).
        wd_tiles = {}

        def issue_wd(hb):
            wda = wd_pool.tile([128, NIT, HB], BF16, tag="wda",
                               name=f"wda{hb}")
            nc.scalar.dma_start(wda[:], wd_d[0, hb])
            wdb = wd_pool.tile([128, NIT, HB], BF16, tag="wdb",
                               name=f"wdb{hb}")
            nc.scalar.dma_start(wdb[:], wd_d[1, hb])
            wsd = wd_pool.tile([128, len(SH_M), HB], BF16, tag="wsd",
                               name=f"wsd{hb}")
            nc.scalar.dma_start(wsd[:], swd_d[hb])
            wd_tiles[hb] = (wda, wdb, wsd)

        pw = xin_pool.tile([128, nt, T], BF16, tag="pw")
        nc.scalar.dma_start(pw[:], pw_d[:])
        issue_wd(0)
        issue_wd(1)

        # ---- stage A: routed experts SwiGLU (gathered slots) ----
        for j, cap, off in ((0, cap_a, 0), (1, cap_b, cap_a)):
            for it in range(NIT):
                wgc = wgu_pool.tile([128, NHC, 128], BF16, tag="wg")
                nc.sync.dma_start(wgc[:], wg_d[j, it])
                wuc = wgu_pool.tile([128, NHC, 128], BF16, tag="wu")
                nc.sync.dma_start(wuc[:], wu_d[j, it])
                psg = psa_pool.tile([128, 512], F32, tag="psg")
                psu = psa_pool.tile([128, 512], F32, tag="psu")
                for hc in range(NHC):
                    nc.tensor.matmul(psg[:, :cap], wgc[:, hc, :],
                                     xgT[:, hc, off:off + cap],
                                     start=(hc == 0), stop=(hc == NHC - 1))
                for hc in range(NHC):
                    nc.tensor.matmul(psu[:, :cap], wuc[:, hc, :],
                                     xgT[:, hc, off:off + cap],
                                     start=(hc == 0), stop=(hc == NHC - 1))
                sg = act_pool.tile([128, 512], F32, tag="sg")
                nc.scalar.activation(sg[:, :cap], psg[:, :cap], ACTF.Silu)
                nc.vector.tensor_mul(ch_rt[it][:, off:off + cap],
                                     sg[:, :cap], psu[:, :cap])

        # ---- stage B: down-projections + scatter combine + ReduceScatter ----
        ccin = [dram_pool.tile([T, HB], BF16, name=f"ccin{v}")
                for v in range(NHB)]
        ccout = [dram_pool.tile([T // NC, HB], BF16, name=f"ccout{v}")
                 for v in range(NHB)]

        for hb in range(NHB):
            wda, wdb, wsd = wd_tiles.pop(hb)

            ys = []
            for ti, (j, off, w) in enumerate(tiles):
                psy = psb_pool.tile([128, HB], F32, tag="psy")
                wdj = wda if j == 0 else wdb
                for it in range(NIT):
                    nc.tensor.matmul(psy[:w], ch_rt[it][:, off:off + w],
                                     wdj[:, it, :],
                                     start=(it == 0), stop=(it == NIT - 1))
                y = y_pool.tile([128, HB], BF16, tag=f"y{ti}")
                nc.scalar.copy(y[:w], psy[:w])
                ys.append(y)

            for tt in range(NTT):
                ts_ = slice(tt * 128, (tt + 1) * 128)
                ps = psb_pool.tile([128, HB], F32, tag="ps")
                n_acc = len(SH_M) + sum(1 for ti in range(nt) if tt in adj[ti])
                k = 0
                for sit, m in enumerate(SH_M):
                    k += 1
                    nc.tensor.matmul(ps[:], ch_sh[sit][:m, ts_],
                                     wsd[:m, sit, :],
                                     start=(k == 1), stop=(k == n_acc))
                for ti, (j, off, w) in enumerate(tiles):
                    if tt not in adj[ti]:
                        continue
                    k += 1
                    nc.tensor.matmul(ps[:], pw[:w, ti, ts_], ys[ti][:w],
                                     start=False, stop=(k == n_acc))
                ob = ob_pool.tile([128, HB], BF16, tag="ob")
                nc.scalar.copy(ob[:], ps[:])
                nc.sync.dma_start(ccin[hb][ts_, :], ob[:])

            if hb + 2 < NHB:
                # all readers of the wd slot being recycled are emitted above
                issue_wd(hb + 2)
            nc.gpsimd.collective_compute(
                "ReduceScatter",
                ALU.add,
                replica_groups=[list(range(NC))],
                ins=[ccin[hb][:].opt()],
                outs=[ccout[hb][:].opt()],
            )
            nc.sync.dma_start(out_d[hb], ccout[hb][:])

    nc.compile()
    nc.m = get_hw_module(nc.m)
    return nc


_PROGRAM = {}


def _get_program(key, cap_a, cap_b, tiles, adj):
    if key not in _PROGRAM:
        _PROGRAM[key] = _build_program(cap_a, cap_b, tiles, adj)
    return _PROGRAM[key]


def kernel(x, gate_w, w_gate, w_up, w_down, sw_gate, sw_up, sw_down,
           _trace=False):
    f = np.float32
    x = np.asarray(x, f)
    comb = _route(x, np.asarray(gate_w, f))
    pairs, cap_a, cap_b, tiles = _plan(comb)
    ns = cap_a + cap_b
    nt = len(tiles)

    xT = np.ascontiguousarray(x.T).astype(BF)                    # [H, T]
    # [NTB, 128, NHC, TB]: xTr[tb, p, hc, t] = x[tb*TB+t, hc*128+p]
    xTr = np.ascontiguousarray(
        xT.reshape(NHC, 128, NTB, TB).transpose(2, 1, 0, 3))

    def tile_wgu(w):  # [I, H] -> [NIT, 128p(h), NHC, 128(i)]
        return np.ascontiguousarray(
            np.asarray(w, f).reshape(NIT, 128, NHC, 128).transpose(0, 3, 2, 1)
        ).astype(BF)

    def tile_wd(w):  # [H, I] -> [NHB, 128p(i), NIT, HB]
        return np.ascontiguousarray(
            np.asarray(w, f).reshape(NHB, HB, NIT, 128).transpose(0, 3, 2, 1)
        ).astype(BF)

    def tile_swgu(w, r):  # [SI, H] slice -> [3(sit), 128p(h), NHC, 128(i)]
        sl = np.asarray(w[SIL * r:SIL * (r + 1)], f)             # [SIL, H]
        pad = np.zeros((len(SH_M) * 128, H), f)
        pad[:SIL] = sl
        return np.ascontiguousarray(
            pad.reshape(len(SH_M), 128, NHC, 128).transpose(0, 3, 2, 1)
        ).astype(BF)

    def tile_swd(w, r):  # [H, SI] slice -> [NHB, 128p(si), 3, HB]
        sl = np.asarray(w[:, SIL * r:SIL * (r + 1)], f)          # [H, SIL]
        pad = np.zeros((H, len(SH_M) * 128), f)
        pad[:, :SIL] = sl
        return np.ascontiguousarray(
            pad.reshape(NHB, HB, len(SH_M), 128).transpose(0, 3, 2, 1)
        ).astype(BF)

    in_maps = []
    pws = []
    for r in range(NC):
        ea, eb = pairs[r]
        xgT = np.zeros((H, ns), BF)
        pw = np.zeros((128, nt, T), BF)
        nta = (cap_a + 127) // 128
        for j, (e, off, tb0) in enumerate(((ea, 0, 0), (eb, cap_a, nta))):
            tok = np.nonzero(comb[:, e])[0]
            cw = comb[tok, e]
            s = np.arange(len(tok))
            xgT[:, off + s] = xT[:, tok]
            pw[s % 128, tb0 + s // 128, tok] = cw.astype(BF)
        pws.append(pw)
        in_maps.append({
            "xTr": xTr,
            "xgT": np.ascontiguousarray(
                xgT.reshape(NHC, 128, ns).transpose(1, 0, 2)),
            "pw": pw,
            "wg": np.stack([tile_wgu(w_gate[ea]), tile_wgu(w_gate[eb])]),
            "wu": np.stack([tile_wgu(w_up[ea]), tile_wgu(w_up[eb])]),
            "wd": np.stack([tile_wd(w_down[ea]), tile_wd(w_down[eb])]),
            "swg": tile_swgu(sw_gate, r),
            "swu": tile_swgu(sw_up, r),
            "swd": tile_swd(sw_down, r),
        })

    # scatter adjacency: union over cores of nonzero 128-token blocks
    adj = []
    for ti in range(nt):
        cols = set()
        for pw in pws:
            blk = pw[:, ti, :].reshape(128, NTT, 128)
            cols.update(np.nonzero(blk.any(axis=(0, 2)))[0].tolist())
        adj.append(sorted(cols))

    key = (cap_a, cap_b, tuple(tuple(a) for a in adj))
    nc = _get_program(key, cap_a, cap_b, tiles, adj)
    res = bass_utils.run_bass_kernel_spmd(
        nc, in_maps, core_ids=list(range(NC)), trace=_trace)

    out = np.empty((T, H), np.float32)
    rows = T // NC
    for r in range(NC):
        o = np.asarray(res.results[r]["out"], np.float32)  # [NHB, rows, HB]
        out[rows * r:rows * (r + 1)] = o.transpose(1, 0, 2).reshape(rows, H)
    if _trace:
        kernel._last_results = res
    return out


# revision 17
# speedup vs baseline: 1.0597x; 1.0597x over previous
"""DeepseekV2 MoE layer on 8 Trainium2 NeuronCores (Bass/Tile).

Strategy (expert-parallel, per sharding hint):
  - Router (softmax + top-6 + renormalize) computed host-side in fp64;
    it is 67 MFLOP of a 106 GFLOP layer but serializes the device
    pipeline, so the host precomputes the dispatch instead:
      * experts greedy-paired 2-per-core to balance token counts,
      * per-core gathered activations x^T[:, selected tokens] are
        pre-tiled host-side (slot-major, expert blocks A|B),
      * combine weights are folded into a per-core scatter one-hot
        P_w[slot, token] (bf16) built host-side.
  - Device per core: SwiGLU over its ~800 slots (bf16, f32 psum),
    shared-expert SwiGLU sharded 8-way over the intermediate dim,
    down-projections + scatter-combine accumulated in one PSUM group
    per token tile, ReduceScatter (bf16) per 512-column block.
  - Scatter matmuls are emitted only for (slot-tile, token-tile) pairs
    that are nonzero on at least one core (slots are token-sorted
    within an expert, so each slot tile touches ~3-4 token tiles).

All weights are pre-tiled host-side into the exact SBUF layouts so
every device DMA is a few large contiguous descriptors.
"""

import numpy as np
import ml_dtypes

import concourse.bass as bass  # noqa: F401  (AP types)
import concourse.mybir as mybir
import concourse.tile as tile
from concourse import bacc
from concourse import bass_utils
from concourse.bass_interp import get_hw_module

F32 = mybir.dt.float32
BF16 = mybir.dt.bfloat16
ALU = mybir.AluOpType
ACTF = mybir.ActivationFunctionType

T = 1024      # tokens
H = 2048      # hidden
I = 1408      # moe intermediate
E = 16        # routed experts
K = 6         # experts per token
SI = 2816     # shared intermediate
NC = 8        # cores
NHC = H // 128           # 16 h chunks
NIT = I // 128           # 11 routed i tiles
SIL = SI // NC           # shared intermediate per core (352)
SH_M = [128, 128, 96]    # shared i tile heights
HB = 512                 # stage-B h block == ReduceScatter chunk
NHB = H // HB            # 4
NTT = T // 128           # 8 token tiles
TB = 512                 # stage-A token block for the shared expert
NTB = T // TB            # 2
BF = ml_dtypes.bfloat16


def _route(x, gate_w):
    """Exact router in fp64: comb[t, e] = renormalized top-6 softmax weight."""
    xl = np.asarray(x, np.float64)
    logits = xl @ np.asarray(gate_w, np.float64).T
    logits -= logits.max(-1, keepdims=True)
    ex = np.exp(logits)
    probs = ex / ex.sum(-1, keepdims=True)
    idx = np.argsort(-probs, axis=-1, kind="stable")[:, :K]
    topw = np.take_along_axis(probs, idx, axis=-1)
    topw = topw / topw.sum(-1, keepdims=True)
    comb = np.zeros((T, E))
    np.put_along_axis(comb, idx, topw, axis=-1)
    return comb


def _plan(comb):
    """Pair experts 2-per-core (big with small) and fix slot capacities."""
    counts = (comb > 0).sum(0)
    order = np.argsort(-counts, kind="stable")
    pairs = [(int(order[r]), int(order[2 * NC - 1 - r])) for r in range(NC)]
    cap_a = int(max(counts[a] for a, _ in pairs))
    cap_b = int(max(counts[b] for _, b in pairs))
    # slot tiles: expert block A at cols [0, cap_a), block B at [cap_a, ns)
    tiles = []  # (j, col_offset, width)
    for j, cap, off in ((0, cap_a, 0), (1, cap_b, cap_a)):
        for t0 in range(0, cap, 128):
            tiles.append((j, off + t0, min(128, cap - t0)))
    return pairs, cap_a, cap_b, tiles


def _build_program(cap_a, cap_b, tiles, adj):
    """adj[ti] = sorted list of token-tile indices with any nonzero P block."""
    ns = cap_a + cap_b
    nt = len(tiles)
    nc = bacc.Bacc("TRN2", target_bir_lowering=False, debug=False,
                   enable_asserts=False, num_devices=NC)

    xTr_d = nc.dram_tensor("xTr", [NTB, 128, NHC, TB], BF16, kind="ExternalInput")
    xgT_d = nc.dram_tensor("xgT", [128, NHC, ns], BF16, kind="ExternalInput")
    pw_d = nc.dram_tensor("pw", [128, nt, T], BF16, kind="ExternalInput")
    wg_d = nc.dram_tensor("wg", [2, NIT, 128, NHC, 128], BF16, kind="ExternalInput")
    wu_d = nc.dram_tensor("wu", [2, NIT, 128, NHC, 128], BF16, kind="ExternalInput")
    wd_d = nc.dram_tensor("wd", [2, NHB, 128, NIT, HB], BF16, kind="ExternalInput")
    swg_d = nc.dram_tensor("swg", [len(SH_M), 128, NHC, 128], BF16,
                           kind="ExternalInput")
    swu_d = nc.dram_tensor("swu", [len(SH_M), 128, NHC, 128], BF16,
                           kind="ExternalInput")
    swd_d = nc.dram_tensor("swd", [NHB, 128, len(SH_M), HB], BF16,
                           kind="ExternalInput")
    out_d = nc.dram_tensor("out", [NHB, T // NC, HB], BF16, kind="ExternalOutput")

    import contextlib
    with tile.TileContext(nc) as tc, contextlib.ExitStack() as st:
        xin_pool = st.enter_context(tc.tile_pool(name="xin", bufs=1))
        ch_pool = st.enter_context(tc.tile_pool(name="ch", bufs=1))
        wgu_pool = st.enter_context(tc.tile_pool(name="wgu", bufs=2))
        wd_pool = st.enter_context(tc.tile_pool(name="wd", bufs=2))
        y_pool = st.enter_context(tc.tile_pool(name="yb", bufs=1))
        act_pool = st.enter_context(tc.tile_pool(name="act", bufs=2))
        ob_pool = st.enter_context(tc.tile_pool(name="ob", bufs=8))
        psa_pool = st.enter_context(tc.tile_pool(name="psa", bufs=2, space="PSUM"))
        psb_pool = st.enter_context(tc.tile_pool(name="psb", bufs=2, space="PSUM"))
        dram_pool = st.enter_context(tc.tile_pool(name="dram", bufs=1, space="DRAM"))

        # ---- resident inputs, ordered so first-needed data lands first ----
        xTr = xin_pool.tile([128, NTB, NHC, TB], BF16, tag="xTr")
        nc.sync.dma_start(xTr[:, 0], xTr_d[0])
        swg = [xin_pool.tile([128, NHC, 128], BF16, tag=f"swg{s}",
                             name=f"swg{s}") for s in range(len(SH_M))]
        swu = [xin_pool.tile([128, NHC, 128], BF16, tag=f"swu{s}",
                             name=f"swu{s}") for s in range(len(SH_M))]
        for s in range(len(SH_M)):
            nc.sync.dma_start(swg[s][:], swg_d[s])
            nc.sync.dma_start(swu[s][:], swu_d[s])
        nc.sync.dma_start(xTr[:, 1], xTr_d[1])
        xgT = xin_pool.tile([128, NHC, ns], BF16, tag="xgT")
        nc.sync.dma_start(xgT[:], xgT_d[:])

        ch_sh = [ch_pool.tile([128, T], BF16, tag=f"chs{s}", name=f"chs{s}")
                 for s in range(len(SH_M))]
        ch_rt = [ch_pool.tile([128, ns], BF16, tag=f"chr{it}", name=f"chr{it}")
                 for it in range(NIT)]

        # ---- stage A: shared expert SwiGLU (all tokens, SIL slice) ----
        for tb in range(NTB):
            for sit, m in enumerate(SH_M):
                psg = psa_pool.tile([128, 512], F32, tag="psg")
                psu = psa_pool.tile([128, 512], F32, tag="psu")
                for hc in range(NHC):
                    nc.tensor.matmul(psg[:m], swg[sit][:, hc, :m],
                                     xTr[:, tb, hc, :],
                                     start=(hc == 0), stop=(hc == NHC - 1))
                for hc in range(NHC):
                    nc.tensor.matmul(psu[:m], swu[sit][:, hc, :m],
                                     xTr[:, tb, hc, :],
                                     start=(hc == 0), stop=(hc == NHC - 1))
                sg = act_pool.tile([128, 512], F32, tag="sg")
                nc.scalar.activation(sg[:m], psg[:m], ACTF.Silu)
                nc.vector.tensor_mul(ch_sh[sit][:m, tb * TB:(tb + 1) * TB],
                                     sg[:m], psu[:m])

        # ---- stage B weight prefetch plumbing ----
        # stage-B weights go through the Activation engine's HWDGE queues so
        # the paced gate/up stream on the SP queues can't head-of-line block
        # them (SP queue entries stall at the head waiting for pool slots).
        wd_tiles = {}

        def issue_wd(hb):
            wda = wd_pool.tile([128, NIT, HB], BF16, tag="wda",
                               name=f"wda{hb}")
            nc.scalar.dma_start(wda[:], wd_d[0, hb])
            wdb = wd_pool.tile([128, NIT, HB], BF16, tag="wdb",
                               name=f"wdb{hb}")
            nc.scalar.dma_start(wdb[:], wd_d[1, hb])
            wsd = wd_pool.tile([128, len(SH_M), HB], BF16, tag="wsd",
                               name=f"wsd{hb}")
            nc.scalar.dma_start(wsd[:], swd_d[hb])
            wd_tiles[hb] = (wda, wdb, wsd)

        pw = xin_pool.tile([128, nt, T], BF16, tag="pw")
        nc.scalar.dma_start(pw[:], pw_d[:])
        issue_wd(0)
        issue_wd(1)

        # ---- stage A: routed experts SwiGLU (gathered slots) ----
        for j, cap, off in ((0, cap_a, 0), (1, cap_b, cap_a)):
            for it in range(NIT):
                wgc = wgu_pool.tile([128, NHC, 128], BF16, tag="wg")
                nc.sync.dma_start(wgc[:], wg_d[j, it])
                wuc = wgu_pool.tile([128, NHC, 128], BF16, tag="wu")
                nc.sync.dma_start(wuc[:], wu_d[j, it])
                psg = psa_pool.tile([128, 512], F32, tag="psg")
                psu = psa_pool.tile([128, 512], F32, tag="psu")
                for hc in range(NHC):
                    nc.tensor.matmul(psg[:, :cap], wgc[:, hc, :],
                                     xgT[:, hc, off:off + cap],
                                     start=(hc == 0), stop=(hc == NHC - 1))
                for hc in range(NHC):
                    nc.tensor.matmul(psu[:, :cap], wuc[:, hc, :],
                                     xgT[:, hc, off:off + cap],
                                     start=(hc == 0), stop=(hc == NHC - 1))
                sg = act_pool.tile([128, 512], F32, tag="sg")
                nc.scalar.activation(sg[:, :cap], psg[:, :cap], ACTF.Silu)
                nc.vector.tensor_mul(ch_rt[it][:, off:off + cap],
                                     sg[:, :cap], psu[:, :cap])

        # ---- stage B: down-projections + scatter combine + ReduceScatter ----
        ccin = [dram_pool.tile([T, HB], BF16, name=f"ccin{v}")
                for v in range(NHB)]
        ccout = [dram_pool.tile([T // NC, HB], BF16, name=f"ccout{v}")
                 for v in range(NHB)]

        for hb in range(NHB):
            wda, wdb, wsd = wd_tiles.pop(hb)

            ys = []
            for ti, (j, off, w) in enumerate(tiles):
                psy = psb_pool.tile([128, HB], F32, tag="psy")
                wdj = wda if j == 0 else wdb
                for it in range(NIT):
                    nc.tensor.matmul(psy[:w], ch_rt[it][:, off:off + w],
                                     wdj[:, it, :],
                                     start=(it == 0), stop=(it == NIT - 1))
                y = y_pool.tile([128, HB], BF16, tag=f"y{ti}")
                nc.scalar.copy(y[:w], psy[:w])
                ys.append(y)

            for tt in range(NTT):
                ts_ = slice(tt * 128, (tt + 1) * 128)
                ps = psb_pool.tile([128, HB], F32, tag="ps")
                n_acc = len(SH_M) + sum(1 for ti in range(nt) if tt in adj[ti])
                k = 0
                for sit, m in enumerate(SH_M):
                    k += 1
                    nc.tensor.matmul(ps[:], ch_sh[sit][:m, ts_],
                                     wsd[:m, sit, :],
                                     start=(k == 1), stop=(k == n_acc))
                for ti, (j, off, w) in enumerate(tiles):
                    if tt not in adj[ti]:
                        continue
                    k += 1
                    nc.tensor.matmul(ps[:], pw[:w, ti, ts_], ys[ti][:w],
                                     start=False, stop=(k == n_acc))
                ob = ob_pool.tile([128, HB], BF16, tag="ob")
                nc.scalar.copy(ob[:], ps[:])
                nc.sync.dma_start(ccin[hb][ts_, :], ob[:])

            if hb + 2 < NHB:
                # all readers of the wd slot being recycled are emitted above
                issue_wd(hb + 2)
            nc.gpsimd.collective_compute(
                "ReduceScatter",
                ALU.add,
                replica_groups=[list(range(NC))],
                ins=[ccin[hb][:].opt()],
                outs=[ccout[hb][:].opt()],
            )
            nc.sync.dma_start(out_d[hb], ccout[hb][:])

    nc.compile()
    nc.m = get_hw_module(nc.m)
    return nc


_PROGRAM = {}


def _get_program(key, cap_a, cap_b, tiles, adj):
    if key not in _PROGRAM:
        _PROGRAM[key] = _build_program(cap_a, cap_b, tiles, adj)
    return _PROGRAM[key]


def kernel(x, gate_w, w_gate, w_up, w_down, sw_gate, sw_up, sw_down,
           _trace=False):
    f = np.float32
    x = np.asarray(x, f)
    comb = _route(x, np.asarray(gate_w, f))
    pairs, cap_a, cap_b, tiles = _plan(comb)
    ns = cap_a + cap_b
    nt = len(tiles)

    xT = np.ascontiguousarray(x.T).astype(BF)                    # [H, T]
    # [NTB, 128, NHC, TB]: xTr[tb, p, hc, t] = x[tb*TB+t, hc*128+p]
    xTr = np.ascontiguousarray(
        xT.reshape(NHC, 128, NTB, TB).transpose(2, 1, 0, 3))

    def tile_wgu(w):  # [I, H] -> [NIT, 128p(h), NHC, 128(i)]
        return np.ascontiguousarray(
            np.asarray(w, f).reshape(NIT, 128, NHC, 128).transpose(0, 3, 2, 1)
        ).astype(BF)

    def tile_wd(w):  # [H, I] -> [NHB, 128p(i), NIT, HB]
        return np.ascontiguousarray(
            np.asarray(w, f).reshape(NHB, HB, NIT, 128).transpose(0, 3, 2, 1)
        ).astype(BF)

    def tile_swgu(w, r):  # [SI, H] slice -> [3(sit), 128p(h), NHC, 128(i)]
        sl = np.asarray(w[SIL * r:SIL * (r + 1)], f)             # [SIL, H]
        pad = np.zeros((len(SH_M) * 128, H), f)
        pad[:SIL] = sl
        return np.ascontiguousarray(
            pad.reshape(len(SH_M), 128, NHC, 128).transpose(0, 3, 2, 1)
        ).astype(BF)

    def tile_swd(w, r):  # [H, SI] slice -> [NHB, 128p(si), 3, HB]
        sl = np.asarray(w[:, SIL * r:SIL * (r + 1)], f)          # [H, SIL]
        pad = np.zeros((H, len(SH_M) * 128), f)
        pad[:, :SIL] = sl
        return np.ascontiguousarray(
            pad.reshape(NHB, HB, len(SH_M), 128).transpose(0, 3, 2, 1)
        ).astype(BF)

    in_maps = []
    pws = []
    for r in range(NC):
        ea, eb = pairs[r]
        xgT = np.zeros((H, ns), BF)
        pw = np.zeros((128, nt, T), BF)
        nta = (cap_a + 127) // 128
        for j, (e, off, tb0) in enumerate(((ea, 0, 0), (eb, cap_a, nta))):
            tok = np.nonzero(comb[:, e])[0]
            cw = comb[tok, e]
            s = np.arange(len(tok))
            xgT[:, off + s] = xT[:, tok]
            pw[s % 128, tb0 + s // 128, tok] = cw.astype(BF)
        pws.append(pw)
        in_maps.append({
            "xTr": xTr,
            "xgT": np.ascontiguousarray(
                xgT.reshape(NHC, 128, ns).transpose(1, 0, 2)),
            "pw": pw,
            "wg": np.stack([tile_wgu(w_gate[ea]), tile_wgu(w_gate[eb])]),
            "wu": np.stack([tile_wgu(w_up[ea]), tile_wgu(w_up[eb])]),
            "wd": np.stack([tile_wd(w_down[ea]), tile_wd(w_down[eb])]),
            "swg": tile_swgu(sw_gate, r),
            "swu": tile_swgu(sw_up, r),
            "swd": tile_swd(sw_down, r),
        })

    # scatter adjacency: union over cores of nonzero 128-token blocks
    adj = []
    for ti in range(nt):
        cols = set()
        for pw in pws:
            blk = pw[:, ti, :].reshape(128, NTT, 128)
            cols.update(np.nonzero(blk.any(axis=(0, 2)))[0].tolist())
        adj.append(sorted(cols))

    key = (cap_a, cap_b, tuple(tuple(a) for a in adj))
    nc = _get_program(key, cap_a, cap_b, tiles, adj)
    res = bass_utils.run_bass_kernel_spmd(
        nc, in_maps, core_ids=list(range(NC)), trace=_trace)

    out = np.empty((T, H), np.float32)
    rows = T // NC
    for r in range(NC):
        o = np.asarray(res.results[r]["out"], np.float32)  # [NHB, rows, HB]
        out[rows * r:rows * (r + 1)] = o.transpose(1, 0, 2).reshape(rows, H)
    if _trace:
        kernel._last_results = res
    return out
